# revision 51
# baseline (speedup 1.0000x reference)
"""Multi-head cross-attention on 8 Trainium2 NeuronCores.

Sharding: data-parallel over batch (2) x tensor-parallel over heads (4 groups
of 4 heads). Core c handles batch c//4, head-group c%4 (a 256-wide slice of
the QKV projection space). Each core computes a partial output-projection
Y_partial = ctx_c @ Wo_c; a ReduceScatter(add) over each batch's 4 cores
leaves each core with a 512-row shard of the summed output, which the host
concatenates.

On-core dataflow (all matmuls in fp32r at full PE rate):
  - x is PE-transposed to d-major (in two d-halves to halve SBUF residency;
    projections accumulate the halves via an SBUF add). Q^T/K^T = W.T @ x^T
    come out j-major, V = x @ Wv comes out s-major -- exactly the operand
    layouts the attention matmuls need, so no other transposes occur.
  - scores are built k-major (S^T) two PSUM banks at a time, exp'd in one
    [128,1024] scalar-engine op (no max subtraction: scores ~ N(0,1)), and
    fed straight into the PV matmul. V carries 64 ones-columns so the softmax
    denominator lands in PSUM partitions 64..127 of the same matmul; a single
    PSUM-to-PSUM tensor divide normalizes while evicting to SBUF.
  - bq/bk are applied on-device (per-partition bias in j-major layout).
    bv/bo commute through softmax/out-projection exactly (softmax rows sum
    to 1), so the host adds bv @ Wo + bo to the final output.
"""

import numpy as np

B, SEQ, D, H, DH = 2, 2048, 1024, 16, 64
N_CORES = 8
GROUPS = 4            # head-groups per batch (cores per batch)
JG = D // GROUPS      # 256 projection dims per core
HPC = H // GROUPS     # 4 heads per core
P = 128

_cached = {}


def _build_program(seq=SEQ, use_f32r=True, with_collective=True,
                   exp_width=1024):
    import concourse.tile as tile
    from concourse import bacc, mybir
    from concourse.masks import make_identity

    F32 = mybir.dt.float32
    MMT = mybir.dt.float32r if use_f32r else mybir.dt.float32

    def mm(x):
        return x.bitcast(MMT)

    # producers of matmul operands must write rounded f32r (walrus birverifier)
    r = mm

    s_chunks = seq // P          # 16  (128-row chunks)
    sb_chunks = seq // 512       # 4   (512-wide blocks)
    sk_chunks = seq // 1024      # 2   (1024-wide attention blocks)
    d_chunks = D // P            # 8
    dh_chunks = d_chunks // 2    # 4   (per d-half)
    j_chunks = JG // P           # 2

    nc = bacc.Bacc("TRN2", target_bir_lowering=False, debug=False,
                   num_devices=N_CORES)

    x1b = nc.dram_tensor("x1b", [seq, D], F32, kind="ExternalInput")
    x2b = nc.dram_tensor("x2b", [seq, D], F32, kind="ExternalInput")
    wq = nc.dram_tensor("wq", [D, JG], F32, kind="ExternalInput")
    wk = nc.dram_tensor("wk", [D, JG], F32, kind="ExternalInput")
    wv = nc.dram_tensor("wv", [D, JG], F32, kind="ExternalInput")
    wo = nc.dram_tensor("wo", [JG, D], F32, kind="ExternalInput")
    bqr = nc.dram_tensor("bqr", [P, j_chunks], F32, kind="ExternalInput")
    bkr = nc.dram_tensor("bkr", [P, j_chunks], F32, kind="ExternalInput")
    y_out = nc.dram_tensor("y_out", [seq // GROUPS, D], F32,
                           kind="ExternalOutput")

    EXP = mybir.ActivationFunctionType.Exp
    DIV = mybir.AluOpType.divide

    with tile.TileContext(nc) as tc:
        with (
            tc.tile_pool(name="consts", bufs=1) as consts,
            tc.tile_pool(name="wqkv", bufs=3) as wqkv_pool,
            tc.tile_pool(name="wop", bufs=1) as wo_pool,
            tc.tile_pool(name="xload", bufs=6) as xload,
            tc.tile_pool(name="xt", bufs=2) as xt_pool,
            tc.tile_pool(name="acts", bufs=1) as acts,
            tc.tile_pool(name="ctp", bufs=2) as ct_pool,
            tc.tile_pool(name="epool", bufs=4) as epool,
            tc.tile_pool(name="small", bufs=2) as small,
            tc.tile_pool(name="ysb", bufs=4) as ysb,
            tc.tile_pool(name="psum_mm", bufs=2, space="PSUM") as psum_mm,
            tc.tile_pool(name="psum_s", bufs=(2 if exp_width == 1024 else 4), space="PSUM") as psum_s,
            tc.tile_pool(name="psum_u", bufs=2, space="PSUM") as psum_u,
            tc.tile_pool(name="dram", bufs=1, space="DRAM") as dram,
        ):
            ident = consts.tile([P, P], F32)
            make_identity(nc, ident)

            def load_weight_cast(wsb, w_dram, n_outer, width, pat):
                # DMA f32 chunks then cast into the f32r operand tile
                for o in range(n_outer):
                    st = ysb.tile([P, 1024], F32, tag="y",
                                  name=f"wst_{wsb.name}_{o}")
                    nc.sync.dma_start(
                        st[:, :width],
                        w_dram.rearrange(pat, p=P)[:, o, :])
                    nc.vector.tensor_copy(r(wsb[:, o, :]), st[:, :width])

            def load_slab(x_dram, sb):
                xts = []
                for q in range(4):
                    xt_ = xload.tile([P, D], F32, tag="xload")
                    nc.sync.dma_start(
                        xt_[:],
                        x_dram[(sb * 4 + q) * P:(sb * 4 + q + 1) * P, :])
                    xts.append(xt_)
                return xts

            def transpose_slab(x_dram, sb, use_act=False, xts=None):
                # x rows [sb*512, (sb+1)*512) x full D -> xT [P, d_chunks, 512]
                # (d-major). In phase A (use_act) the idle 2-bank score slots
                # hold 8 batched PE transposes evicted by ONE [128,1024] copy,
                # alternating ACT/DVE; during attention (x1) fall back to
                # single-bank "mm" tiles so the score slots stay free.
                if xts is None:
                    xts = load_slab(x_dram, sb)
                xT = xt_pool.tile([P, d_chunks, 512], F32, tag="xT")
                if use_act:
                    for dg in range(d_chunks // 2):
                        pt = psum_s.tile([P, 1024], F32, tag="s",
                                         name=f"ptx_{x_dram.name}_{sb}_{dg}")
                        for i in range(2):
                            dc = 2 * dg + i
                            for q in range(4):
                                nc.tensor.transpose(
                                    pt[:, i * 512 + q * P:
                                       i * 512 + (q + 1) * P],
                                    xts[q][:, dc * P:(dc + 1) * P], ident[:])
                        out2 = xT[:, 2 * dg:2 * dg + 2, :]
                        if dg % 2 == 1:
                            nc.scalar.copy(r(out2), pt[:])
                        else:
                            nc.vector.tensor_copy(r(out2), pt[:])
                else:
                    for dc in range(d_chunks):
                        pt = psum_mm.tile([P, 512], F32, tag="mm")
                        for q in range(4):
                            nc.tensor.transpose(
                                pt[:, q * P:(q + 1) * P],
                                xts[q][:, dc * P:(dc + 1) * P], ident[:])
                        nc.vector.tensor_copy(r(xT[:, dc, :]), pt[:])
                return xT

            # x2 slab 0 loads go first so transposes start immediately;
            # weight DMAs stream in behind them
            x2tiles0 = load_slab(x2b, 0)

            # qkv weights rotate through 2 shared slots (k, v, then q)
            wk_sb = wqkv_pool.tile([P, d_chunks, JG], F32, tag="wqkv")
            wv_sb = wqkv_pool.tile([P, d_chunks, JG], F32, tag="wqkv")
            wo_sb = wo_pool.tile([P, j_chunks, D], F32, tag="wo")
            load_weight_cast(wk_sb, wk, d_chunks, JG, "(o p) j -> p o j")
            load_weight_cast(wv_sb, wv, d_chunks, JG, "(o p) j -> p o j")
            load_weight_cast(wo_sb, wo, j_chunks, D, "(o p) n -> p o n")
            bq_sb = consts.tile([P, j_chunks], F32, tag="bq")
            bk_sb = consts.tile([P, j_chunks], F32, tag="bk")
            nc.sync.dma_start(bq_sb[:], bqr[:])
            nc.sync.dma_start(bk_sb[:], bkr[:])


            kT = acts.tile([P, j_chunks, seq], F32, tag="kT")
            qT = acts.tile([P, j_chunks, seq], F32, tag="qT")
            # V'' per head-column-block: cols 0..63 V_h, 64..127 ones
            vpp = acts.tile([P, s_chunks, HPC * P], F32, tag="vpp")

            ones_f32 = consts.tile([P, DH], F32, tag="ones")
            nc.vector.memset(ones_f32[:], 1.0)
            for si in range(s_chunks):
                ones_view = vpp[:, si].rearrange("p (h q) -> p h q", q=P)[:, :, DH:P]
                nc.vector.tensor_copy(
                    r(ones_view),
                    ones_f32[:, None, :].to_broadcast([P, HPC, DH]))

            def project_jmajor(xT_s, w_sb, sb, out, bias, use_act=False,
                               on_s=False):
                # out[j, sb-slab] = w.T @ xT_s + bias. on_s borrows the
                # attention score PSUM banks (idle before the first exp) so
                # projections pipeline in parallel with the next slab's
                # transposes instead of contending for the 2 "mm" slots.
                ssl = slice(sb * 512, (sb + 1) * 512)
                for jc in range(j_chunks):
                    if on_s:
                        pk = psum_s.tile([P, 512], F32, tag="s",
                                         name=f"pk_{w_sb.name}_{sb}_{jc}")
                    else:
                        pk = psum_mm.tile([P, 512], F32, tag="mm")
                    for dc in range(d_chunks):
                        nc.tensor.matmul(
                            pk[:],
                            mm(w_sb[:, dc, jc * P:(jc + 1) * P]),
                            mm(xT_s[:, dc, :]),
                            start=(dc == 0), stop=(dc == d_chunks - 1))
                    if use_act:
                        nc.scalar.add(r(out[:, jc, ssl]), pk[:],
                                      bias[:, jc:jc + 1])
                    else:
                        nc.vector.tensor_scalar_add(
                            r(out[:, jc, ssl]), pk[:], bias[:, jc:jc + 1])

            def project_v(xT_s, sb):
                # V[s-slab, j] = x2_slab @ Wv into the vpp head blocks
                for q in range(4):
                    si = sb * 4 + q
                    pv = psum_u.tile([P, JG], F32, tag="u")
                    for dc in range(d_chunks):
                        nc.tensor.matmul(
                            pv[:],
                            mm(xT_s[:, dc, q * P:(q + 1) * P]),
                            mm(wv_sb[:, dc, :]),
                            start=(dc == 0), stop=(dc == d_chunks - 1))
                    vv = vpp[:, si].rearrange("p (h q) -> p h q", q=P)[:, :, 0:DH]
                    nc.vector.tensor_copy(
                        r(vv), pv[:].rearrange("p (h q) -> p h q", q=DH))

            ybounce = dram.tile([seq, D], F32, tag="yin")
            yscatter = dram.tile([seq // GROUPS, D], F32, tag="yout")

            cts = {}
            pus_by = {}

            def emit_oproj(sc, cT):
                for s8 in range(8):
                  with nc.named_scope("oproj"):
                    si = sc * 8 + s8
                    yt = ysb.tile([P, D], F32, tag="y",
                                  name=f"yt_{sc}_{s8}")
                    last = sc == sk_chunks - 1
                    for nck in range(2):
                        if last and (s8 * 2 + nck) % 2 == 1:
                            py = psum_s.tile([P, 512], F32, tag="s",
                                             name=f"py_{sc}_{s8}_{nck}")
                        else:
                            py = psum_mm.tile([P, 512], F32, tag="mm",
                                              name=f"py_{sc}_{s8}_{nck}")
                        for jc in range(j_chunks):
                            nc.tensor.matmul(
                                py[:],
                                mm(cT[:, jc, s8 * P:(s8 + 1) * P]),
                                mm(wo_sb[:, jc, nck * 512:(nck + 1) * 512]),
                                start=(jc == 0), stop=(jc == j_chunks - 1))
                        if last:
                            nc.scalar.copy(
                                yt[:, nck * 512:(nck + 1) * 512], py[:])
                        else:
                            nc.vector.tensor_copy(
                                yt[:, nck * 512:(nck + 1) * 512], py[:])
                    nc.sync.dma_start(ybounce[si * P:(si + 1) * P, :], yt[:])

            def emit_pv(sc, h, kc, et):
                jc, po = h // 2, (h % 2) * DH
                if kc == 0:
                    pus_by[(sc, h)] = [
                        psum_u.tile([P, 512], F32, tag="u",
                                    name=f"pu_{sc}_{h}_{i}")
                        for i in range(2)]
                pus = pus_by[(sc, h)]
                for half in range(2):
                    fsl = slice(half * 512, (half + 1) * 512)
                    nc.tensor.matmul(
                        pus[half][:],
                        mm(vpp[:, kc, h * P:(h + 1) * P]),
                        mm(et[:, fsl]),
                        start=(kc == 0), stop=(kc == s_chunks - 1))
                if kc == s_chunks - 1:
                    cT = cts[sc]
                    for half in range(2):
                        fsl = slice(half * 512, (half + 1) * 512)
                        rt = small.tile([DH, 512], F32, tag="rt",
                                        name=f"rt_{sc}_{h}_{half}")
                        nc.vector.reciprocal(rt[:], pus[half][DH:P, :])
                        nc.vector.tensor_mul(
                            r(cT[po:po + DH, jc, fsl]),
                            pus[half][0:DH, :], rt[:])
                    del pus_by[(sc, h)]
                    if h == HPC - 1:
                        emit_oproj(sc, cT)

            pend = []

            def emit_attn_unit(sc, h, kc):
              with nc.named_scope("attn"):
                if (h, kc) == (0, 0):
                    cts[sc] = ct_pool.tile([P, j_chunks, 1024], F32,
                                           tag="cT", name=f"cT_{sc}")
                jc, po = h // 2, (h % 2) * DH
                ps = psum_s.tile([P, 1024], F32, tag="s",
                                 name=f"ps_{sc}_{h}_{kc}")
                for half in range(2):
                    hsl = slice(sc * 1024 + half * 512,
                                sc * 1024 + (half + 1) * 512)
                    nc.tensor.matmul(
                        ps[:, half * 512:(half + 1) * 512],
                        mm(kT[po:po + DH, jc, kc * P:(kc + 1) * P]),
                        mm(qT[po:po + DH, jc, hsl]),
                        start=True, stop=True)
                et = epool.tile([P, 1024], F32, tag="e",
                                name=f"et_{sc}_{h}_{kc}")
                nc.scalar.activation(r(et[:]), ps[:], EXP, scale=0.125)
                pend.append((sc, h, kc, et))
                if len(pend) > 2:
                    emit_pv(*pend.pop(0))

            # ---- x2 -> K^T, V'' (per 512-row slab) ----
            for sb in range(sb_chunks):
                with nc.named_scope("x2t"):
                    x2T_s = transpose_slab(x2b, sb, use_act=True,
                                           xts=(x2tiles0 if sb == 0 else None))
                with nc.named_scope("kproj"):
                    project_jmajor(x2T_s, wk_sb, sb, kT, bk_sb, use_act=True)
                with nc.named_scope("vproj"):
                    project_v(x2T_s, sb)

            # ---- x1 -> Q^T (per slab; overlaps with attention below) ----
            wq_sb = wqkv_pool.tile([P, d_chunks, JG], F32, tag="wqkv")
            load_weight_cast(wq_sb, wq, d_chunks, JG, "(o p) j -> p o j")
            for sb in range(sb_chunks):
                with nc.named_scope("x1t"):
                    x1T_s = transpose_slab(x1b, sb)
                with nc.named_scope("qproj"):
                    project_jmajor(x1T_s, wq_sb, sb, qT, bq_sb, on_s=(sb < 2))

            # ---- attention units (flat, PV lagging exp by 2) ----
            for sc in range(sk_chunks):
                for h in range(HPC):
                    for kc in range(s_chunks):
                        emit_attn_unit(sc, h, kc)
            with nc.named_scope("attn"):
                for args in pend:
                    emit_pv(*args)

            # ---- sum partials across the 4 cores of this batch ----
            if with_collective:
                nc.gpsimd.collective_compute(
                    "ReduceScatter",
                    mybir.AluOpType.add,
                    replica_groups=[[0, 1, 2, 3], [4, 5, 6, 7]],
                    ins=[ybounce[:].opt()],
                    outs=[yscatter[:].opt()],
                )
                nc.sync.dma_start(y_out[:], yscatter[:])
            else:
                nc.sync.dma_start(y_out[:], ybounce[:seq // GROUPS, :])

    nc.compile()
    return nc


def _get_program(seq=SEQ, use_f32r=True):
    key = (seq, use_f32r)
    if key not in _cached:
        _cached[key] = _build_program(seq, use_f32r)
    return _cached[key]


def make_in_maps(x1, x2, Wq, bq, Wk, bk, Wv, bv, Wo, bo):
    """Per-core input dicts for the SPMD program."""
    in_maps = []
    for c in range(N_CORES):
        b, g = c // GROUPS, c % GROUPS
        js = slice(g * JG, (g + 1) * JG)
        in_maps.append({
            "x1b": np.ascontiguousarray(x1[b]),
            "x2b": np.ascontiguousarray(x2[b]),
            "wq": np.ascontiguousarray(Wq[:, js]),
            "wk": np.ascontiguousarray(Wk[:, js]),
            "wv": np.ascontiguousarray(Wv[:, js]),
            "wo": np.ascontiguousarray(Wo[js, :]),
            "bqr": np.ascontiguousarray(bq[js].reshape(2, P).T),
            "bkr": np.ascontiguousarray(bk[js].reshape(2, P).T),
        })
    return in_maps


def assemble(results, Wv_bias_fix):
    """results: list of per-core {'y_out': [seq//GROUPS, D]}"""
    seq = results[0]["y_out"].shape[0] * GROUPS
    Y = np.empty((B, seq, D), np.float32)
    for c in range(N_CORES):
        b, rr = c // GROUPS, c % GROUPS
        rows = slice(rr * (seq // GROUPS), (rr + 1) * (seq // GROUPS))
        Y[b, rows, :] = results[c]["y_out"]
    Y += Wv_bias_fix
    return Y


def kernel(x1, x2, Wq, bq, Wk, bk, Wv, bv, Wo, bo):
    from concourse.bass_utils import run_bass_kernel_spmd

    x1 = np.asarray(x1, np.float32)
    x2 = np.asarray(x2, np.float32)
    Wq, bq = np.asarray(Wq, np.float32), np.asarray(bq, np.float32)
    Wk, bk = np.asarray(Wk, np.float32), np.asarray(bk, np.float32)
    Wv, bv = np.asarray(Wv, np.float32), np.asarray(bv, np.float32)
    Wo, bo = np.asarray(Wo, np.float32), np.asarray(bo, np.float32)

    nc = _get_program(SEQ)
    in_maps = make_in_maps(x1, x2, Wq, bq, Wk, bk, Wv, bv, Wo, bo)
    res = run_bass_kernel_spmd(nc, in_maps, core_ids=list(range(N_CORES)))
    fix = (bv @ Wo + bo).astype(np.float32)
    return assemble(res.results, fix)
